# revision 45
# baseline (speedup 1.0000x reference)
"""Multi-head attention (B=2, S=4096, D=512, H=8) on 8 TRN2 NeuronCores.

Sharding: core c handles batch c//4 and query rows (c%4)*1024 .. +1024 —
each core runs the full attention (all 8 heads) for its query block; the
host only concatenates the 8 output shards.

Final design (vs the 406 us baseline):
  * scores matmuls run as row-tiled PAIRS (heads 2i/2i+1 at PE tile
    positions (0,0)/(64,0)) so the dk=64 contraction no longer idles half
    the array; PV matmuls run as col-tiled pairs (M=64 each, ctx^T halves
    of one PSUM bank); softmax denominators Z come from four concurrent
    M=1 ones-matmuls (col groups 0/32/64/96 of one bank);
  * softmax exp is split 57/43 between the ACT engine (true exp, rescale
    folded into the activation's scale immediate) and a custom 8-stage DVE
    instruction computing ((x+c1)^2+c0)^32 ~ exp(x/c2)
    (importance-weighted fit; adds <2e-3 end-to-end error);
  * score PSUM tiles are [128,512] singles in a 4-deep ring (pipeline
    slack beats larger-chunk exp overhead); V-projection and normalize
    ctx copies run on ACT (they gate SBUF/PSUM ring releases that
    exp-busy DVE would delay); z/y copies, reciprocals and normalize
    muls/adds on DVE; partition broadcasts on Pool (single library);
  * the final O-projection drain alternates PSUM pools to pipeline.
"""

from contextlib import ExitStack

import numpy as np

import concourse.bass as bass
import concourse.tile as tile
from concourse import bacc, mybir
from concourse.bass_utils import run_bass_kernel_spmd

D = 512
H = 8
DK = 64
F32 = mybir.dt.float32
BF16 = mybir.dt.bfloat16
EXP = mybir.ActivationFunctionType.Exp

# ((c2*s + c1)^2 + c0)^32 ~ exp(s); importance-weighted fit over s in
# [-11.5, 10.2] (rel err <0.3% where softmax weight lives, s >= 4).
C2_FIT = 0.024253183821620825
C1_FIT = 0.6338569119227282
C0_FIT = 0.5987354341163744


def _register_dve_exp():
    """Register the custom DVE op (idempotent)."""
    import concourse.dve_ops as dve_ops
    from concourse.dve_spec import Spec, Src0, C0, C1, lower, sq
    from concourse.dve_uop import DveOpSpec

    for op in dve_ops.OPS:
        if op.name == "EXP_Q32_ANT":
            return op

    body = sq(Src0 + C1) + C0
    for _ in range(5):
        body = sq(body)

    def _ref(in0, in1, s0, s1, imm2):
        x = in0.astype(np.float32)
        q = ((x + s1) * (x + s1) + s0).astype(np.float32)
        for _ in range(5):
            q = (q * q).astype(np.float32)
        return q

    spec = Spec(body=body, reference=_ref)
    name = "EXP_Q32_ANT"
    opcode = dve_ops._CUSTOM_DVE_ROW_BASE + len(dve_ops.OPS)
    shas = {}
    for ver in ("v3", "v4"):
        uops = lower(spec, ver=ver)
        shas[ver] = DveOpSpec(
            name=name, opcode=opcode, uops=uops, rd1_en=False
        ).sha(ver)
    op = dve_ops.DveOp(name, spec, subdim=False, uops_sha=shas)
    dve_ops.OPS.append(op)
    dve_ops._SUB_OPCODE_FOR_NAME[name] = opcode
    return op


EXP_Q32 = _register_dve_exp()


def build(T=1024, S=4096, n_cores=8, dve_mod=7, dve_keep=3, debug_taps=False,
          sim_collapse=False, vst_on_act=True, kp_on_act=False,
          early_start=False, ctxcopy_on_act=True,
          sp_bufs=4, cp_bufs=2, zp_bufs=1):
    # sim_collapse: emit ONE matmul per tile-position-concurrent slot so
    # TimelineSim (which does not model PE tile concurrency) approximates
    # real hardware timing. Output is semantically wrong — timing only.
    FC = D // 128     # 4 feature chunks
    SC = S // 128     # 32 key chunks
    NW = T // 512     # 2 query windows
    PW = 2048         # raw k/v staging piece width
    NP = S // PW

    nc = bacc.Bacc("TRN2", target_bir_lowering=False, debug=False,
                   num_devices=n_cores)

    qT = nc.dram_tensor("qT", [D, T], BF16, kind="ExternalInput").ap()
    kT = nc.dram_tensor("kT", [D, S], BF16, kind="ExternalInput").ap()
    vT = nc.dram_tensor("vT", [D, S], BF16, kind="ExternalInput").ap()
    wqT8 = nc.dram_tensor("wqT8", [D, D], BF16, kind="ExternalInput").ap()
    wkT = nc.dram_tensor("wkT", [D, D], BF16, kind="ExternalInput").ap()
    wvT = nc.dram_tensor("wvT", [D, D], BF16, kind="ExternalInput").ap()
    woT = nc.dram_tensor("woT", [D, D], BF16, kind="ExternalInput").ap()
    bq8 = nc.dram_tensor("bq8", [D, 1], F32, kind="ExternalInput").ap()
    bk = nc.dram_tensor("bk", [D, 1], F32, kind="ExternalInput").ap()
    bv = nc.dram_tensor("bv", [D, 1], F32, kind="ExternalInput").ap()
    bo = nc.dram_tensor("bo", [1, D], F32, kind="ExternalInput").ap()
    y = nc.dram_tensor("y", [T, D], F32, kind="ExternalOutput").ap()
    if debug_taps:
        dbg_qt = nc.dram_tensor("dbg_qt", [128, T], F32, kind="ExternalOutput").ap()
        dbg_kt = nc.dram_tensor("dbg_kt", [128, 1024], F32, kind="ExternalOutput").ap()
        dbg_vst = nc.dram_tensor("dbg_vst", [128, D], F32, kind="ExternalOutput").ap()
        dbg_z = nc.dram_tensor("dbg_z", [128, 512], F32, kind="ExternalOutput").ap()
        dbg_ctx = nc.dram_tensor("dbg_ctx", [128, 512], F32, kind="ExternalOutput").ap()
        dbg_pt = nc.dram_tensor("dbg_pt", [128, 1024], F32, kind="ExternalOutput").ap()

    act_scale = float(1.0 / C2_FIT)

    with tile.TileContext(nc) as tc, ExitStack() as ctx:
        const = ctx.enter_context(tc.tile_pool(name="const", bufs=1))
        qtp = ctx.enter_context(tc.tile_pool(name="qtp", bufs=1))
        ctxp = ctx.enter_context(tc.tile_pool(name="ctxp", bufs=1))
        ktp = ctx.enter_context(tc.tile_pool(name="ktp", bufs=1))
        vstp = ctx.enter_context(tc.tile_pool(name="vstp", bufs=SC))
        rawp = ctx.enter_context(tc.tile_pool(name="rawp", bufs=12))
        ptp = ctx.enter_context(tc.tile_pool(name="ptp", bufs=5))
        ctxsp = ctx.enter_context(tc.tile_pool(name="ctxsp", bufs=4))
        rbp = ctx.enter_context(tc.tile_pool(name="rbp", bufs=4))
        smallp = ctx.enter_context(tc.tile_pool(name="smallp", bufs=6))
        yp = ctx.enter_context(tc.tile_pool(name="yp", bufs=2))
        ps_sp = ctx.enter_context(tc.tile_pool(name="ps_sp", bufs=sp_bufs, space="PSUM"))
        ps_cp = ctx.enter_context(tc.tile_pool(name="ps_cp", bufs=cp_bufs, space="PSUM"))
        ps_zp = ctx.enter_context(tc.tile_pool(name="ps_zp", bufs=zp_bufs, space="PSUM"))
        ps_gp = ctx.enter_context(tc.tile_pool(name="ps_gp", bufs=1, space="PSUM"))

        # ---- constants ----
        wk_t, wv_t, wo_t = [], [], []
        for f in range(FC):
            t = const.tile([128, D], BF16, name=f"wk{f}", tag=f"wk{f}")
            nc.sync.dma_start(t[:], wkT[f * 128:(f + 1) * 128, :])
            wk_t.append(t)
            t = const.tile([128, D], BF16, name=f"wv{f}", tag=f"wv{f}")
            nc.sync.dma_start(t[:], wvT[f * 128:(f + 1) * 128, :])
            wv_t.append(t)
            t = const.tile([128, D], BF16, name=f"wo{f}", tag=f"wo{f}")
            nc.sync.dma_start(t[:], woT[f * 128:(f + 1) * 128, :])
            wo_t.append(t)
        bo_t = const.tile([1, D], F32, name="bo_t", tag="bo_t")
        nc.sync.dma_start(bo_t[:], bo[:])
        ones_t = const.tile([1, 128], F32, name="ones_t", tag="ones_t")
        nc.vector.memset(ones_t[:], 1.0)
        onesb = const.tile([128, 1], BF16, name="onesb", tag="onesb")
        nc.vector.memset(onesb[:], 1.0)
        bq_t, bk_t, bv_t = [], [], []
        for f in range(FC):
            t = const.tile([128, 1], F32, name=f"bq{f}", tag=f"bq{f}")
            nc.sync.dma_start(t[:], bq8[f * 128:(f + 1) * 128, :])
            bq_t.append(t)
            t = const.tile([128, 1], F32, name=f"bkc{f}", tag=f"bkc{f}")
            nc.sync.dma_start(t[:], bk[f * 128:(f + 1) * 128, :])
            bk_t.append(t)
        for h in range(H):
            t = const.tile([64, 1], F32, name=f"bvh{h}", tag=f"bvh{h}")
            nc.sync.dma_start(t[:], bv[h * 64:(h + 1) * 64, :])
            bv_t.append(t)

        # ---- Q projection: QT[hc] = [128, T] (rows = head-pair dims) ----
        QT_t = [qtp.tile([128, T], BF16, name=f"QT{h}", tag=f"QT{h}")
                for h in range(FC)]
        CTX_t = [ctxp.tile([128, T], BF16, name=f"CTX{h}", tag=f"CTX{h}")
                 for h in range(FC)]
        with tc.tile_pool(name="wqp", bufs=1) as wqp:
            wq_t = []
            for f in range(FC):
                t = wqp.tile([128, D], BF16, name=f"wq{f}", tag=f"wq{f}")
                nc.sync.dma_start(t[:], wqT8[f * 128:(f + 1) * 128, :])
                wq_t.append(t)
            qraw = []
            for f in range(FC):
                t = rawp.tile([128, T], BF16, name=f"qraw{f}", tag="raw")
                nc.sync.dma_start(t[:], qT[f * 128:(f + 1) * 128, :])
                qraw.append(t)
            for hc in range(FC):
                for w in range(NW):
                    ps = ps_sp.tile([128, 512], F32, name="ps_q", tag="pss")
                    for f in range(FC):
                        nc.tensor.matmul(
                            ps[:],
                            wq_t[f][:, hc * 128:(hc + 1) * 128],
                            qraw[f][:, w * 512:(w + 1) * 512],
                            start=(f == 0), stop=(f == FC - 1))
                    nc.vector.tensor_scalar_add(
                        QT_t[hc][:, w * 512:(w + 1) * 512], ps[:],
                        bq_t[hc][:])

        # ---- deferred projection work units ----
        vst_all = [None] * SC
        _vraw = {}

        def v_round(r):
            p, tl = divmod(r, PW // 128)
            if tl == 0:
                _vraw[p] = []
                for f in range(FC):
                    t = rawp.tile([128, PW], BF16, name=f"vraw{f}", tag="raw")
                    nc.sync.dma_start(
                        t[:], vT[f * 128:(f + 1) * 128, p * PW:(p + 1) * PW])
                    _vraw[p].append(t)
            vraw = _vraw[p]
            vst = vstp.tile([128, D], BF16, name="vst", tag="vst")
            ps = ps_gp.tile([128, D], F32, name="ps_v", tag="psg")
            for f in range(FC):
                nc.tensor.matmul(
                    ps[:],
                    vraw[f][:, tl * 128:(tl + 1) * 128],
                    wv_t[f][:],
                    start=(f == 0), stop=(f == FC - 1))
            if vst_on_act:
                nc.scalar.copy(vst[:], ps[:])
            else:
                nc.vector.tensor_copy(vst[:], ps[:])
            vst_all[r] = vst

        _v_done = [0]

        def ensure_v(chunk):
            while _v_done[0] <= min(chunk, SC - 1):
                v_round(_v_done[0])
                _v_done[0] += 1

        _kraw = {}
        kt_t = [ktp.tile([128, S], BF16, name=f"kt{i}", tag=f"kt{i}")
                for i in range(FC)]

        KR = FC * (S // 512)  # 32 rounds total
        # reorder rounds so raw pieces are loaded per piece for hc pairs:
        # do (hc0, hc1) over all windows first, then (hc2, hc3).
        k_order = []
        for half in range(2):
            for p in range(NP):
                for hc in (2 * half, 2 * half + 1):
                    for wi in range(PW // 512):
                        k_order.append((hc, p, wi))

        _k_done = [0]

        def k_round2(idx):
            hc, p, wi = k_order[idx]
            key = (hc // 2, p)  # reload raw K per hc-half so slots recycle
            if key not in _kraw:
                _kraw[key] = []
                for f in range(FC):
                    t = rawp.tile([128, PW], BF16, name=f"kraw{f}", tag="raw")
                    nc.sync.dma_start(
                        t[:], kT[f * 128:(f + 1) * 128, p * PW:(p + 1) * PW])
                    _kraw[key].append(t)
            kraw = _kraw[key]
            ps = ps_gp.tile([128, 512], F32, name="ps_k", tag="psg")
            for f in range(FC):
                nc.tensor.matmul(
                    ps[:],
                    wk_t[f][:, hc * 128:(hc + 1) * 128],
                    kraw[f][:, wi * 512:(wi + 1) * 512],
                    start=(f == 0), stop=(f == FC - 1))
            kslice = kt_t[hc][:, p * PW + wi * 512:p * PW + (wi + 1) * 512]
            if kp_on_act:
                nc.scalar.add(kslice, ps[:], bk_t[hc][:])
            else:
                nc.vector.tensor_scalar_add(kslice, ps[:], bk_t[hc][:])

        def ensure_k(n):
            while _k_done[0] < min(n, KR):
                k_round2(_k_done[0])
                _k_done[0] += 1

        # ---- O projection round (one 128-row t-chunk) ----
        def o_round(ti, alt=False):
            if alt:
                ps_y = ps_sp.tile([128, 512], F32, name="ps_y2", tag="pss")
            else:
                ps_y = ps_gp.tile([128, D], F32, name="ps_y", tag="psg")
            for f in range(FC):
                nc.tensor.matmul(
                    ps_y[:],
                    CTX_t[f][:, ti * 128:(ti + 1) * 128],
                    wo_t[f][:],
                    start=(f == 0), stop=False)
            nc.tensor.matmul(ps_y[:], ones_t[0:1, :], bo_t[0:1, :],
                             start=False, stop=True)
            yt = yp.tile([128, D], F32, name="yt", tag="y")
            nc.vector.tensor_copy(yt[:], ps_y[:])
            nc.sync.dma_start(y[ti * 128:(ti + 1) * 128, :], yt[:])

        # ---- prelude: kt for quad 0 (first piece only if early_start) ----
        ensure_k(KR // 4 if early_start else KR // 2)
        ensure_v(3)

        if debug_taps:
            dbgp = ctx.enter_context(tc.tile_pool(name="dbgp", bufs=1))
            t = dbgp.tile([128, T], F32, name="dq", tag="dq")
            nc.vector.tensor_copy(t[:], QT_t[0][:])
            nc.sync.dma_start(dbg_qt[:], t[:])
            t = dbgp.tile([128, 1024], F32, name="dk", tag="dk")
            nc.vector.tensor_copy(t[:], kt_t[0][:, 0:1024])
            nc.sync.dma_start(dbg_kt[:], t[:])
            t = dbgp.tile([128, D], F32, name="dv", tag="dv")
            nc.vector.tensor_copy(t[:], vst_all[0][:])
            nc.sync.dma_start(dbg_vst[:], t[:])

        # ---- attention ----
        chunk_idx = [0]
        pend_o = []

        for w in range(NW):
            for g in range(2):
                ctx_ps = [
                    ps_cp.tile([128, 512], F32, name="ctxAB", tag="ctx"),
                    ps_cp.tile([128, 512], F32, name="ctxCD", tag="ctx"),
                ]
                z_ps = ps_zp.tile([128, 512], F32, name="zq", tag="z")
                def emit_scores(sc):
                    pts = []
                    for pair in range(2):  # AB, CD
                        hc = 2 * g + pair
                        pt = ptp.tile([128, 1024], BF16, name="pt", tag="pt")
                        ps_s0 = None
                        for j in range(2):
                            if j == 0 or not sim_collapse:
                                ps_s = ps_sp.tile([128, 512], F32,
                                                  name="ps_s", tag="pss")
                                nc.tensor.matmul(
                                    ps_s[:],
                                    kt_t[hc][j * 64:(j + 1) * 64,
                                             sc * 128:(sc + 1) * 128],
                                    QT_t[hc][j * 64:(j + 1) * 64,
                                             w * 512:(w + 1) * 512],
                                    start=True, stop=True)
                                ps_s0 = ps_s
                            else:
                                ps_s = ps_s0
                            ci = chunk_idx[0]
                            chunk_idx[0] += 1
                            if ci % dve_mod < dve_keep:
                                nc.vector._custom_dve(
                                    EXP_Q32,
                                    out=pt[:, j * 512:(j + 1) * 512],
                                    in0=ps_s[:], s0=C0_FIT, s1=C1_FIT)
                            else:
                                nc.scalar.activation(
                                    pt[:, j * 512:(j + 1) * 512], ps_s[:],
                                    EXP, scale=act_scale)
                        if debug_taps and w == 0 and g == 0 and sc == 0 \
                                and pair == 0:
                            t = dbgp.tile([128, 1024], F32, name="dp",
                                          tag="dp")
                            nc.vector.tensor_copy(t[:], pt[:])
                            nc.sync.dma_start(dbg_pt[:], t[:])
                        pts.append(pt)
                    return pts

                def emit_pvz(sc, pts):
                    for pair in range(2):
                        hc = 2 * g + pair
                        pt = pts[pair]
                        for j in range(1 if sim_collapse else 2):
                            h = 2 * hc + j
                            nc.tensor.matmul(
                                ctx_ps[pair][j * 64:(j + 1) * 64, :],
                                vst_all[sc][:, h * 64:(h + 1) * 64],
                                pt[:, j * 512:(j + 1) * 512],
                                start=(sc == 0), stop=(sc == SC - 1))
                    for pair in range(1 if sim_collapse else 2):
                        pt = pts[pair]
                        for j in range(1 if sim_collapse else 2):
                            hl = 2 * pair + j
                            nc.tensor.matmul(
                                z_ps[32 * hl:32 * hl + 1, :],
                                onesb[:],
                                pt[:, j * 512:(j + 1) * 512],
                                start=(sc == 0), stop=(sc == SC - 1),
                                tile_position=(0, 32 * hl))

                # software-pipelined by one iteration: scores(sc+1) are
                # issued before PV/Z(sc) so the exp of iteration sc has a
                # full extra PE iteration of latency slack
                pts_prev = emit_scores(0)
                for sc in range(SC):
                    # interleave deferred projection / O rounds
                    if w == 0 and g == 0:
                        if early_start:
                            ensure_k(KR // 4 + sc + 1)
                        elif sc % 2 == 0:
                            ensure_k(KR // 2 + sc // 2 + 1)
                        ensure_v(sc + 4)
                    if w == 1 and g == 0 and sc % 8 == 0 and pend_o:
                        o_round(pend_o.pop(0))
                    if sc + 1 < SC:
                        pts_next = emit_scores(sc + 1)
                    emit_pvz(sc, pts_prev)
                    pts_prev = pts_next
                # ---- normalize the quad (all tensor-op inputs base-0) ----
                if debug_taps and w == 0 and g == 0:
                    t = dbgp.tile([128, 512], F32, name="dz", tag="dz")
                    nc.vector.tensor_copy(t[:], z_ps[:])
                    nc.sync.dma_start(dbg_z[:], t[:])
                    t = dbgp.tile([128, 512], F32, name="dc", tag="dc")
                    nc.vector.tensor_copy(t[:], ctx_ps[0][:])
                    nc.sync.dma_start(dbg_ctx[:], t[:])
                for hl in range(4):
                    hc = 2 * g + hl // 2
                    h = 4 * g + hl
                    rows = slice((hl % 2) * 64, (hl % 2) * 64 + 64)
                    if sim_collapse:
                        rows = slice(0, 64)
                    t = ctxsp.tile([64, 512], F32, name="ctxs", tag="ctxs")
                    if ctxcopy_on_act:
                        nc.scalar.copy(t[:], ctx_ps[hl // 2][rows, :])
                    else:
                        nc.vector.tensor_copy(t[:], ctx_ps[hl // 2][rows, :])
                    zrow = 0 if sim_collapse else 32 * hl
                    zr = smallp.tile([1, 512], F32, name="zr", tag="zr")
                    nc.vector.tensor_copy(zr[:], z_ps[zrow:zrow + 1, :])
                    r = smallp.tile([1, 512], F32, name="r", tag="r")
                    nc.vector.reciprocal_approx_fast(r[:], zr[:])
                    rb = rbp.tile([64, 512], F32, name="rb", tag="rb")
                    nc.gpsimd.partition_broadcast(rb[:], r[0:1, :])
                    nc.vector.tensor_mul(t[:], t[:], rb[:])
                    cslice = CTX_t[hc][rows, w * 512:(w + 1) * 512]
                    nc.vector.tensor_scalar_add(cslice, t[:], bv_t[h][:])
            pend_o.extend(range(w * (T // 128) // NW,
                                (w + 1) * (T // 128) // NW))
        alt = False
        while pend_o:
            o_round(pend_o.pop(0), alt=alt)
            alt = not alt

    nc.compile()
    return nc


_CACHE = {}


def _get_compiled():
    if "nc" not in _CACHE:
        _CACHE["nc"] = build(T=1024, S=4096, n_cores=8)
    return _CACHE["nc"]


def make_in_maps(q, k, v, W_q, b_q, W_k, b_k, W_v, b_v, W_o, b_o, n_cores=8):
    import ml_dtypes
    bf = ml_dtypes.bfloat16
    f = np.float32
    qsc = f(C2_FIT) / np.sqrt(f(DK))
    qT = [np.ascontiguousarray(np.asarray(q[b], f).T.astype(bf)) for b in range(q.shape[0])]
    kT = [np.ascontiguousarray(np.asarray(k[b], f).T.astype(bf)) for b in range(k.shape[0])]
    vT = [np.ascontiguousarray(np.asarray(v[b], f).T.astype(bf)) for b in range(v.shape[0])]
    shared = {
        "wqT8": np.ascontiguousarray((np.asarray(W_q, f).T * qsc).astype(bf)),
        "wkT": np.ascontiguousarray(np.asarray(W_k, f).T.astype(bf)),
        "wvT": np.ascontiguousarray(np.asarray(W_v, f).T.astype(bf)),
        "woT": np.ascontiguousarray(np.asarray(W_o, f).T.astype(bf)),
        "bq8": np.asarray(b_q, f).reshape(D, 1) * qsc,
        "bk": np.asarray(b_k, f).reshape(D, 1),
        "bv": np.asarray(b_v, f).reshape(D, 1),
        "bo": np.asarray(b_o, f).reshape(1, D),
    }
    n_b = q.shape[0]
    blocks_per_b = n_cores // n_b
    T = q.shape[1] // blocks_per_b
    in_maps = []
    for c in range(n_cores):
        b, wdx = divmod(c, blocks_per_b)
        m = dict(shared)
        m["qT"] = np.ascontiguousarray(qT[b][:, wdx * T:(wdx + 1) * T])
        m["kT"] = kT[b]
        m["vT"] = vT[b]
        in_maps.append(m)
    return in_maps


def kernel(q, k, v, W_q, b_q, W_k, b_k, W_v, b_v, W_o, b_o):
    nc = _get_compiled()
    in_maps = make_in_maps(q, k, v, W_q, b_q, W_k, b_k, W_v, b_v, W_o, b_o)
    res = run_bass_kernel_spmd(nc, in_maps, list(range(8)))
    B, S_full = q.shape[0], q.shape[1]
    T = S_full // (8 // B)
    out = np.empty((B, S_full, D), np.float32)
    for c in range(8):
        b, wdx = divmod(c, 8 // B)
        out[b, wdx * T:(wdx + 1) * T, :] = res.results[c]["y"]
    return out
